# revision 51
# baseline (speedup 1.0000x reference)
"""GAT layer kernel for Trainium2, 8-core data-parallel over batch.

Math (per batch b, head h):
    h = x @ W                              [N, H*HD]
    s_n = <h[n, h*HD:(h+1)*HD], a_src[h]>  t_n likewise with a_dst
    A[j, i] = exp(leakyrelu(s_i + t_j, 0.2))
    out[i]  = (sum_j A[j, i] * h_j) / (sum_j A[j, i])

Key identity: softmax columns are scale-invariant, so drop the e^{s_i}
column factor entirely:
    A'[j, i] = A[j, i] * e^{-s_i} = max(e^{t_j}, e^{0.2 t_j} * u_i),
    u_i = e^{-0.8 s_i}
Both numerator and Z pick up the same e^{-s_i}, which cancels in the
division. Each [128, N] attention tile is then ONE vector tensor_scalar
op (in0 = broadcast u row, two per-partition scalar columns e^{t_j} /
e^{0.2 t_j}, ops mult+max) running in the DVE's 4x packed mode -- vs the
2 ACT / 2 DVE ops per tile of the direct formulation.

Per core (= one batch element):
  - st rows = (W @ a_ext)^T @ xT via one small f32r matmul (host
    precomputes Wa = W @ a_ext); u row = Exp(-0.8 s) on ACT.
  - u broadcast [1,N] -> [128,N]: head 0 (ramp-critical) via a K=1
    ones-column outer product on the PE + a DVE PSUM->SBUF copy; heads
    1-3 via DRAM-round-trip broadcast DMAs (queue bandwidth, off the
    critical path).
  - h_node blocks AND transposed t columns from the SAME per-j-tile
    matmul by extending its rhs to [W | Wa | pad] (256 f32r cols: at
    >=256 columns the PE streams f32r at full rate, and the pad lands
    exactly in the PSUM tile's bank-aligned pitch). The per-j-tile
    e^{t}/e^{0.2 t} scalar columns and bf16 weight tiles are carved out
    per j-tile on the otherwise-idle ACT engine so the first head never
    waits on the full projection.
  - main loop per (head, j-tile): one tensor_scalar -> A' tile (bf16),
    two 512-col matmuls accumulate [h_node | ones]^T @ A' into PSUM
    [33, N] (row 32 = Z'). The PE pipelines back-to-back matmuls at
    ~216 ns/512 cols, so DVE A'-tile production is the limiter; GpSimd
    produces the last j-tile of heads 1-3 to keep DVE ahead.
  - tail: Z rows -> z8 [8, 512] (rows = (h, half)) via ACT copy + tiny
    fold DMAs, nrz = -1/Z via ONE fused xor+add int-magic seed op and
    one Newton step (3 DVE ops total), one K=8 indicator matmul per
    512-half broadcasts nrz to all 128 (h,d) output rows, one fused
    (num * -1) * nrz multiply per half, DMA out. Numerator rows are
    copied PSUM->SBUF per head on ACT during the main loop so PSUM
    banks are free for the broadcast matmuls.
  - host transposes out^T back to node-major when unsharding.
"""

import numpy as np

B, N, IN_F, OUT_F, H = 8, 1024, 128, 128, 4
HD = OUT_F // H  # 32
NEG = 0.2
N_CORES = 8
NT = N // 128  # 8 node tiles
CW = 256  # hnst per-j-tile column pitch (= padded rhs width)

A_DT = "bfloat16"  # dtype of the attention tiles + matmul weights

_CACHE = {}


def _build_nc():
    import concourse.bacc as bacc
    import concourse.tile as tile
    from concourse import mybir

    f32 = mybir.dt.float32
    f32r = mybir.dt.float32r
    i32 = mybir.dt.int32
    adt = getattr(mybir.dt, A_DT)
    AF = mybir.ActivationFunctionType
    ALU = mybir.AluOpType

    nc = bacc.Bacc("TRN2", target_bir_lowering=False, debug=False,
                   num_devices=N_CORES)

    xT = nc.declare_dram_parameter("xT", [IN_F, N], f32, isOutput=False)
    Wd = nc.declare_dram_parameter("W", [IN_F, OUT_F], f32, isOutput=False)
    Wa = nc.declare_dram_parameter("Wa", [IN_F, 2 * H], f32, isOutput=False)
    ind97_d = nc.declare_dram_parameter("ind97", [3 * HD + 1, OUT_F], adt,
                                        isOutput=False)
    outT = nc.declare_dram_parameter("outT", [OUT_F, N], adt, isOutput=True)

    u_dram = nc.dram_tensor("u_scratch", [H, N], adt)

    with tile.TileContext(nc) as tc:
      with (
        tc.tile_pool(name="const", bufs=1) as cpool,
        tc.tile_pool(name="atile", bufs=12) as apool,
        tc.tile_pool(name="tail", bufs=1) as tpool,
      ):
        with (
            tc.tile_pool(name="ps_pre", bufs=1, space="PSUM") as pspre,
            tc.tile_pool(name="ps_ub", bufs=1, space="PSUM") as psub,
        ):
            # ---- load inputs: W/Wa first on their queues (small, and
            # they gate the st/hnst chains), then the xT halves; second
            # xT half issued by the Scalar engine's queue ----
            xT_sb = cpool.tile([IN_F, N], f32, tag="xT")
            WW = cpool.tile([IN_F, OUT_F + 2 * H], f32, tag="WW")
            nc.sync.dma_start(out=WW[:, OUT_F:], in_=Wa[:])
            nc.scalar.dma_start(out=WW[:, 0:OUT_F], in_=Wd[:])
            nc.sync.dma_start(out=xT_sb[:, 0:512], in_=xT[:, 0:512])
            nc.scalar.dma_start(out=xT_sb[:, 512:N], in_=xT[:, 512:N])
            # ind97 ships pre-cast as bf16: an on-device cast would sit
            # in the DVE FIFO gated on this (late, off-critical) DMA and
            # stall the ramp-critical xTr/War4 casts behind it
            ind97 = cpool.tile([3 * HD + 1, OUT_F], adt, tag="ind97")
            nc.gpsimd.dma_start(out=ind97, in_=ind97_d[:])

            # ones row for the K=1 u-broadcast outer product
            ones1 = cpool.tile([1, 128], adt, tag="ones1")
            nc.gpsimd.memset(ones1, 1.0)

            # f32r casts (fp32 matmul is 1/4 rate); W/Wa casts first --
            # they land ~3us before xT -- then xT halves so the c0 ramp
            # chain starts half a transfer earlier
            War4 = cpool.tile([IN_F, H], f32r, tag="War4")
            nc.vector.tensor_copy(out=War4, in_=WW[:, OUT_F:OUT_F + H])
            WWr = cpool.tile([IN_F, CW], f32r, tag="WWr")
            nc.gpsimd.memset(WWr[:, OUT_F + 2 * H:].bitcast(f32), 0.0)
            nc.vector.tensor_copy(out=WWr[:, 0:OUT_F + 2 * H], in_=WW)
            xTr = cpool.tile([IN_F, N], f32r, tag="xTr")
            nc.vector.tensor_copy(out=xTr[:, 0:512], in_=xT_sb[:, 0:512])
            nc.vector.tensor_copy(out=xTr[:, 512:N], in_=xT_sb[:, 512:N])


            # ---- fully 512-column-chunked ramp: per half c, the chain
            # st matmul -> uexp -> u_dram/outer-product -> u_b0 copy
            # starts as soon as that half of xT has landed, interleaved
            # on the PE with the projection matmuls whose lhsT columns
            # that half covers ----
            st_ps = pspre.tile([H, N], f32, tag="st")
            u_rows = cpool.tile([H, N], adt, tag="u_rows")
            hnst = pspre.tile([128, NT * CW], f32, tag="hnst")
            hnst_v = hnst[:].rearrange("p (jt c) -> p jt c", c=CW)
            ub_ps = psub.tile([128, N], f32, tag="ub0ps")
            u_b0 = cpool.tile([128, N], adt, tag="ub0")
            u_b = [u_b0]
            for h in range(1, H):
                u_b.append(cpool.tile([128, N], adt, name=f"ub{h}",
                                      tag=f"ub{h}"))

            etc = cpool.tile([128, H * NT], f32, tag="etc")
            etc02 = cpool.tile([128, H * NT], f32, tag="etc02")
            tcols = hnst_v[:, :, OUT_F + H:OUT_F + 2 * H]
            etc_v = etc[:].rearrange("p (h jt) -> p jt h", jt=NT)
            etc02_v = etc02[:].rearrange("p (h jt) -> p jt h", jt=NT)
            wt_all = cpool.tile([128, NT * 33 * H], adt, tag="wt")
            wt_v = wt_all[:].rearrange("p (jt h c) -> p jt h c", h=H, c=33)
            nc.gpsimd.memset(wt_v[:, :, :, 32:33], 1.0)
            hn_v = hnst_v[:, :, 0:OUT_F].rearrange(
                "p jt (h d) -> p jt h d", d=HD)

            def half_chain(c):
                cs = slice(512 * c, 512 * (c + 1))
                nc.tensor.matmul(st_ps[:, cs], War4, xTr[:, cs],
                                 start=True, stop=True)
                nc.scalar.activation(out=u_rows[:, cs], in_=st_ps[:, cs],
                                     func=AF.Exp, scale=-0.8)
                nc.sync.dma_start(out=u_dram[:, cs], in_=u_rows[:, cs])
                # heads 1-3 u broadcast halves launch as soon as the
                # corresponding u_dram half is written
                for h in range(1, H):
                    eng = nc.gpsimd if h == 2 else nc.sync
                    eng.dma_start(
                        out=u_b[h][:, cs],
                        in_=u_dram[h:h + 1, cs].to_broadcast([128, 512]))
                for jt in (0, 1) if c == 0 else (2, 3):
                    nc.tensor.matmul(
                        hnst[:, CW * jt:CW * (jt + 1)],
                        xTr[:, 128 * jt:128 * (jt + 1)],
                        WWr, start=True, stop=True)
                # head-0 u broadcast: K=1 outer product + bf16 copy-out
                nc.tensor.matmul(ub_ps[:, cs], ones1, u_rows[0:1, cs],
                                 start=True, stop=True)
                nc.vector.tensor_copy(out=u_b0[:, cs], in_=ub_ps[:, cs])

            # ---- t scalar columns + weight tiles, batched on ACT in
            # chunks (per-op ACT overhead dominates tiny ops, but one
            # batch would gate the first A' tile on the last projection
            # matmul); jt0-1 ops slot between the two uexp halves ----
            half_chain(0)
            nc.scalar.activation(out=etc_v[:, 0:2], in_=tcols[:, 0:2],
                                 func=AF.Exp)
            nc.scalar.activation(out=etc02_v[:, 0:2], in_=tcols[:, 0:2],
                                 func=AF.Exp, scale=NEG)
            nc.scalar.copy(out=wt_v[:, 0:1, :, 0:32], in_=hn_v[:, 0:1])
            half_chain(1)
            for jt in range(4, NT):
                nc.tensor.matmul(
                    hnst[:, CW * jt:CW * (jt + 1)],
                    xTr[:, 128 * jt:128 * (jt + 1)],
                    WWr, start=True, stop=True)
            nc.scalar.activation(out=etc_v[:, 2:], in_=tcols[:, 2:],
                                 func=AF.Exp)
            nc.scalar.activation(out=etc02_v[:, 2:], in_=tcols[:, 2:],
                                 func=AF.Exp, scale=NEG)
            nc.scalar.copy(out=wt_v[:, 1:, :, 0:32], in_=hn_v[:, 1:])
            wts = [wt_all[:, 132 * jt:132 * (jt + 1)] for jt in range(NT)]

        # num4[32h+d, i] = unnormalized out rows; zq row 32h = Z of head
        # h (rows between are dead lanes: engines can only address
        # 32-aligned partition offsets, and NR over the dead lanes is
        # free -- DVE cost is free-dim-driven. Memset keeps them 0 so
        # the K=97 broadcast matmul sees finite values under weight 0.)
        num4 = tpool.tile([128, N], f32, tag="num4")
        zq = tpool.tile([3 * HD + 1, N], f32, tag="zq")
        nc.gpsimd.memset(zq, 0.0)

        # ---- main loop: one tensor_scalar + two matmuls per (h, jt).
        # Heads 0/1 interleave at jt granularity 4: their jt0-3 tiles
        # depend only on the early scalar carve, so DVE never idles
        # waiting for the jt4-7 carve (which needs the last projection
        # matmuls) ----
        with tc.tile_pool(name="ps_main", bufs=1, space="PSUM") as psmain:
            ohs = [psmain.tile([33, N], f32, name=f"oh{h}", tag=f"oh{h}")
                   for h in range(H)]

            def run_tiles(h, jts):
                oh = ohs[h]
                for jt in jts:
                    idx = h * NT + jt
                    a_t = apool.tile([128, N], adt, name=f"at{idx}",
                                     tag="at")
                    # all on DVE: gpsimd's tensor_scalar is ~30x slower
                    # and its SBUF traffic degrades concurrent DVE ops.
                    # The very first tile goes in halves behind the
                    # half-copies of u_b0.
                    spans = [(0, 512), (512, N)] if idx == 0 else [(0, N)]
                    for lo, hi in spans:
                        nc.vector.tensor_scalar(
                            out=a_t[:, lo:hi], in0=u_b[h][:, lo:hi],
                            scalar1=etc02[:, idx:idx + 1],
                            scalar2=etc[:, idx:idx + 1],
                            op0=ALU.mult, op1=ALU.max)
                    for c in range(2):
                        nc.tensor.matmul(
                            oh[:, 512 * c:512 * (c + 1)],
                            wts[jt][:, 33 * h:33 * (h + 1)],
                            a_t[:, 512 * c:512 * (c + 1)],
                            start=(jt == 0), stop=(jt == NT - 1))

            def head_epilogue(h):
                # pipelined with the next heads' bulk work, on the
                # otherwise-idle ACT engine: Z row straight to partition
                # 32h (32-aligned = legal), numerator rows to num4
                # (frees the PSUM banks for the rzb matmuls)
                oh = ohs[h]
                for c in range(2):
                    nc.scalar.copy(
                        out=zq[HD * h:HD * h + 1, 512 * c:512 * (c + 1)],
                        in_=oh[32:33, 512 * c:512 * (c + 1)])
                nc.scalar.copy(out=num4[HD * h:HD * (h + 1), :],
                               in_=oh[0:32, :])

            run_tiles(0, range(0, 4))
            run_tiles(1, range(0, 4))
            run_tiles(0, range(4, NT))
            head_epilogue(0)
            run_tiles(1, range(4, NT))
            head_epilogue(1)
            for h in (2, 3):
                run_tiles(h, range(NT))
                head_epilogue(h)

        # ---- tail: nrz = -1/Z via int-magic seed + one Newton step
        # (xor and add cannot fuse: HW rejects mixed bitwise/arith) ----
        NP = 3 * HD + 1
        ynot = tpool.tile([NP, N], f32, tag="ynot")
        y = tpool.tile([NP, N], f32, tag="y")
        m = tpool.tile([NP, N], f32, tag="nr_m")
        # bf16 nrz: the rzb matmuls then stream at full bf16 rate with
        # overlapped weight loads; ~0.4% scale error vs the 2e-2 gate
        nrz = tpool.tile([NP, N], adt, tag="nrz")
        # per column half, so half 0's broadcast/multiply/DMA overlap
        # half 1's Newton chain
        for c in range(2):
            cs = slice(512 * c, 512 * (c + 1))
            nc.vector.tensor_scalar(
                out=ynot[:, cs].bitcast(i32), in0=zq[:, cs].bitcast(i32),
                scalar1=0xFFFFFFFF - (1 << 32), scalar2=None,
                op0=ALU.bitwise_xor)
            nc.vector.tensor_scalar(
                out=y[:, cs].bitcast(i32), in0=ynot[:, cs].bitcast(i32),
                scalar1=0x7EF311C4, scalar2=None, op0=ALU.add)
            nc.vector.tensor_tensor(out=m[:, cs], in0=zq[:, cs],
                                    in1=y[:, cs], op=ALU.mult)
            nc.vector.scalar_tensor_tensor(
                out=nrz[:, cs], in0=m[:, cs], scalar=2.0, in1=y[:, cs],
                op0=ALU.subtract, op1=ALU.mult)  # (m-2)*y = -1/Z
        with tc.tile_pool(name="ps_norm", bufs=1, space="PSUM") as psnorm:
            for c in range(2):
                # rzb[32h+d, i'] = nrz[32h, i'] via K=97 indicator matmul
                rzb = psnorm.tile([128, 512], f32, tag=f"rzb{c}")
                nc.tensor.matmul(rzb[:, :], ind97,
                                 nrz[:, 512 * c:512 * (c + 1)],
                                 start=True, stop=True)
                o_sb = tpool.tile([128, 512], adt, tag=f"osb{c}")
                nc.vector.scalar_tensor_tensor(
                    out=o_sb, in0=num4[:, 512 * c:512 * (c + 1)],
                    scalar=-1.0, in1=rzb, op0=ALU.mult, op1=ALU.mult)
                eng = nc.sync if c == 0 else nc.gpsimd
                eng.dma_start(out=outT[:, 512 * c:512 * (c + 1)], in_=o_sb)

    nc.compile()
    return nc


def _get_nc():
    if "nc" not in _CACHE:
        _CACHE["nc"] = _build_nc()
    return _CACHE["nc"]


def make_in_maps(x, W, a_src, a_dst):
    a_ext = np.zeros((OUT_F, 2 * H), np.float32)
    for h in range(H):
        a_ext[h * HD:(h + 1) * HD, h] = a_src[h]
        a_ext[h * HD:(h + 1) * HD, H + h] = a_dst[h]
    Wa = W @ a_ext
    # ind97[k, p] = 1 iff k == 32*(p//32) (Z-row broadcast), pre-cast
    # to bf16 so the device needs no conversion
    import ml_dtypes
    ind97 = np.zeros((3 * HD + 1, OUT_F), ml_dtypes.bfloat16)
    for h in range(H):
        ind97[HD * h, HD * h:HD * (h + 1)] = 1.0
    return [
        {"xT": np.ascontiguousarray(x[c].T), "W": W, "Wa": Wa,
         "ind97": ind97}
        for c in range(N_CORES)
    ]


def kernel(x, W, a_src, a_dst):
    from concourse.bass_utils import run_bass_kernel_spmd

    x = np.asarray(x, dtype=np.float32)
    W = np.asarray(W, dtype=np.float32)
    a_src = np.asarray(a_src, dtype=np.float32)
    a_dst = np.asarray(a_dst, dtype=np.float32)

    nc = _get_nc()
    in_maps = make_in_maps(x, W, a_src, a_dst)
    res = run_bass_kernel_spmd(nc, in_maps, core_ids=list(range(N_CORES)))
    out = np.stack([np.asarray(res.results[c]["outT"]).astype(np.float32).T
                    for c in range(N_CORES)], axis=0)
    return np.ascontiguousarray(out, dtype=np.float32)


# revision 52
# speedup vs baseline: 1.0234x; 1.0234x over previous
"""GAT layer kernel for Trainium2, 8-core data-parallel over batch.

Math (per batch b, head h):
    h = x @ W                              [N, H*HD]
    s_n = <h[n, h*HD:(h+1)*HD], a_src[h]>  t_n likewise with a_dst
    A[j, i] = exp(leakyrelu(s_i + t_j, 0.2))
    out[i]  = (sum_j A[j, i] * h_j) / (sum_j A[j, i])

Key identity: softmax columns are scale-invariant, so drop the e^{s_i}
column factor entirely:
    A'[j, i] = A[j, i] * e^{-s_i} = max(e^{t_j}, e^{0.2 t_j} * u_i),
    u_i = e^{-0.8 s_i}
Both numerator and Z pick up the same e^{-s_i}, which cancels in the
division. Each [128, N] attention tile is then ONE vector tensor_scalar
op (in0 = broadcast u row, two per-partition scalar columns e^{t_j} /
e^{0.2 t_j}, ops mult+max) running in the DVE's 4x packed mode -- vs the
2 ACT / 2 DVE ops per tile of the direct formulation.

Per core (= one batch element):
  - st rows = (W @ a_ext)^T @ xT via one small f32r matmul (host
    precomputes Wa = W @ a_ext); u row = Exp(-0.8 s) on ACT.
  - u broadcast [1,N] -> [128,N]: head 0 (ramp-critical) via a K=1
    ones-column outer product on the PE + a DVE PSUM->SBUF copy; heads
    1-3 via DRAM-round-trip broadcast DMAs (queue bandwidth, off the
    critical path).
  - h_node blocks AND transposed t columns from the SAME per-j-tile
    matmul by extending its rhs to [W | Wa | pad] (256 f32r cols: at
    >=256 columns the PE streams f32r at full rate, and the pad lands
    exactly in the PSUM tile's bank-aligned pitch). The per-j-tile
    e^{t}/e^{0.2 t} scalar columns and bf16 weight tiles are carved out
    per j-tile on the otherwise-idle ACT engine so the first head never
    waits on the full projection.
  - main loop per (head, j-tile): one tensor_scalar -> A' tile (bf16),
    two 512-col matmuls accumulate [h_node | ones]^T @ A' into PSUM
    [33, N] (row 32 = Z'). The PE pipelines back-to-back matmuls at
    ~216 ns/512 cols, so DVE A'-tile production is the limiter; GpSimd
    produces the last j-tile of heads 1-3 to keep DVE ahead.
  - tail: Z rows -> z8 [8, 512] (rows = (h, half)) via ACT copy + tiny
    fold DMAs, nrz = -1/Z via ONE fused xor+add int-magic seed op and
    one Newton step (3 DVE ops total), one K=8 indicator matmul per
    512-half broadcasts nrz to all 128 (h,d) output rows, one fused
    (num * -1) * nrz multiply per half, DMA out. Numerator rows are
    copied PSUM->SBUF per head on ACT during the main loop so PSUM
    banks are free for the broadcast matmuls.
  - host transposes out^T back to node-major when unsharding.
"""

import numpy as np

B, N, IN_F, OUT_F, H = 8, 1024, 128, 128, 4
HD = OUT_F // H  # 32
NEG = 0.2
N_CORES = 8
NT = N // 128  # 8 node tiles
CW = 256  # hnst per-j-tile column pitch (= padded rhs width)

A_DT = "bfloat16"  # dtype of the attention tiles + matmul weights

_CACHE = {}


def _build_nc():
    import concourse.bacc as bacc
    import concourse.tile as tile
    from concourse import mybir

    f32 = mybir.dt.float32
    f32r = mybir.dt.float32r
    i32 = mybir.dt.int32
    adt = getattr(mybir.dt, A_DT)
    AF = mybir.ActivationFunctionType
    ALU = mybir.AluOpType

    nc = bacc.Bacc("TRN2", target_bir_lowering=False, debug=False,
                   num_devices=N_CORES)

    xT = nc.declare_dram_parameter("xT", [IN_F, N], f32, isOutput=False)
    Wd = nc.declare_dram_parameter("W", [IN_F, OUT_F], f32, isOutput=False)
    Wa = nc.declare_dram_parameter("Wa", [IN_F, 2 * H], f32, isOutput=False)
    ind97_d = nc.declare_dram_parameter("ind97", [3 * HD + 1, OUT_F], adt,
                                        isOutput=False)
    outT = nc.declare_dram_parameter("outT", [OUT_F, N], adt, isOutput=True)

    u_dram = nc.dram_tensor("u_scratch", [H, N], adt)

    with tile.TileContext(nc) as tc:
      with (
        tc.tile_pool(name="const", bufs=1) as cpool,
        tc.tile_pool(name="atile", bufs=12) as apool,
        tc.tile_pool(name="tail", bufs=1) as tpool,
      ):
        with (
            tc.tile_pool(name="ps_pre", bufs=1, space="PSUM") as pspre,
            tc.tile_pool(name="ps_ub", bufs=1, space="PSUM") as psub,
        ):
            # ---- load inputs: W/Wa first on their queues (small, and
            # they gate the st/hnst chains), then the xT halves; second
            # xT half issued by the Scalar engine's queue ----
            xT_sb = cpool.tile([IN_F, N], f32, tag="xT")
            WW = cpool.tile([IN_F, OUT_F + 2 * H], f32, tag="WW")
            nc.sync.dma_start(out=WW[:, OUT_F:], in_=Wa[:])
            nc.scalar.dma_start(out=WW[:, 0:OUT_F], in_=Wd[:])
            nc.sync.dma_start(out=xT_sb[:, 0:512], in_=xT[:, 0:512])
            nc.scalar.dma_start(out=xT_sb[:, 512:N], in_=xT[:, 512:N])
            # ind97 ships pre-cast as bf16: an on-device cast would sit
            # in the DVE FIFO gated on this (late, off-critical) DMA and
            # stall the ramp-critical xTr/War4 casts behind it
            ind97 = cpool.tile([3 * HD + 1, OUT_F], adt, tag="ind97")
            nc.gpsimd.dma_start(out=ind97, in_=ind97_d[:])

            # ones row for the K=1 u-broadcast outer product
            ones1 = cpool.tile([1, 128], adt, tag="ones1")
            nc.gpsimd.memset(ones1, 1.0)

            # f32r casts (fp32 matmul is 1/4 rate); W/Wa casts first --
            # they land ~3us before xT -- then xT halves so the c0 ramp
            # chain starts half a transfer earlier
            War4 = cpool.tile([IN_F, H], f32r, tag="War4")
            nc.vector.tensor_copy(out=War4, in_=WW[:, OUT_F:OUT_F + H])
            WWr = cpool.tile([IN_F, CW], f32r, tag="WWr")
            nc.gpsimd.memset(WWr[:, OUT_F + 2 * H:].bitcast(f32), 0.0)
            nc.vector.tensor_copy(out=WWr[:, 0:OUT_F + 2 * H], in_=WW)
            xTr = cpool.tile([IN_F, N], f32r, tag="xTr")
            nc.vector.tensor_copy(out=xTr[:, 0:512], in_=xT_sb[:, 0:512])
            nc.vector.tensor_copy(out=xTr[:, 512:N], in_=xT_sb[:, 512:N])


            # ---- fully 512-column-chunked ramp: per half c, the chain
            # st matmul -> uexp -> u_dram/outer-product -> u_b0 copy
            # starts as soon as that half of xT has landed, interleaved
            # on the PE with the projection matmuls whose lhsT columns
            # that half covers ----
            st_ps = pspre.tile([H, N], f32, tag="st")
            u_rows = cpool.tile([H, N], adt, tag="u_rows")
            hnst = pspre.tile([128, NT * CW], f32, tag="hnst")
            hnst_v = hnst[:].rearrange("p (jt c) -> p jt c", c=CW)
            ub_ps = psub.tile([128, N], f32, tag="ub0ps")
            u_b0 = cpool.tile([128, N], adt, tag="ub0")
            u_b = [u_b0]
            for h in range(1, H):
                u_b.append(cpool.tile([128, N], adt, name=f"ub{h}",
                                      tag=f"ub{h}"))

            etc = cpool.tile([128, H * NT], f32, tag="etc")
            etc02 = cpool.tile([128, H * NT], f32, tag="etc02")
            tcols = hnst_v[:, :, OUT_F + H:OUT_F + 2 * H]
            etc_v = etc[:].rearrange("p (h jt) -> p jt h", jt=NT)
            etc02_v = etc02[:].rearrange("p (h jt) -> p jt h", jt=NT)
            wt_all = cpool.tile([128, NT * 33 * H], adt, tag="wt")
            wt_v = wt_all[:].rearrange("p (jt h c) -> p jt h c", h=H, c=33)
            nc.gpsimd.memset(wt_v[:, :, :, 32:33], 1.0)
            hn_v = hnst_v[:, :, 0:OUT_F].rearrange(
                "p jt (h d) -> p jt h d", d=HD)

            def half_chain(c):
                cs = slice(512 * c, 512 * (c + 1))
                nc.tensor.matmul(st_ps[:, cs], War4, xTr[:, cs],
                                 start=True, stop=True)
                nc.scalar.activation(out=u_rows[:, cs], in_=st_ps[:, cs],
                                     func=AF.Exp, scale=-0.8)
                nc.sync.dma_start(out=u_dram[:, cs], in_=u_rows[:, cs])
                for jt in (0, 1) if c == 0 else (2, 3):
                    nc.tensor.matmul(
                        hnst[:, CW * jt:CW * (jt + 1)],
                        xTr[:, 128 * jt:128 * (jt + 1)],
                        WWr, start=True, stop=True)
                # head-0 u broadcast: K=1 outer product + bf16 copy-out
                nc.tensor.matmul(ub_ps[:, cs], ones1, u_rows[0:1, cs],
                                 start=True, stop=True)
                nc.vector.tensor_copy(out=u_b0[:, cs], in_=ub_ps[:, cs])

            # ---- t scalar columns + weight tiles, batched on ACT in
            # chunks (per-op ACT overhead dominates tiny ops, but one
            # batch would gate the first A' tile on the last projection
            # matmul); jt0-1 ops slot between the two uexp halves ----
            half_chain(0)
            nc.scalar.activation(out=etc_v[:, 0:2], in_=tcols[:, 0:2],
                                 func=AF.Exp)
            nc.scalar.activation(out=etc02_v[:, 0:2], in_=tcols[:, 0:2],
                                 func=AF.Exp, scale=NEG)
            nc.scalar.copy(out=wt_v[:, 0:1, :, 0:32], in_=hn_v[:, 0:1])
            half_chain(1)
            # heads 1-3 u broadcast via DRAM round trip on the DMA queues
            for h in range(1, H):
                eng = nc.gpsimd if h == 2 else nc.sync
                eng.dma_start(
                    out=u_b[h],
                    in_=u_dram[h:h + 1, :].to_broadcast([128, N]))
            for jt in range(4, NT):
                nc.tensor.matmul(
                    hnst[:, CW * jt:CW * (jt + 1)],
                    xTr[:, 128 * jt:128 * (jt + 1)],
                    WWr, start=True, stop=True)
            nc.scalar.activation(out=etc_v[:, 2:], in_=tcols[:, 2:],
                                 func=AF.Exp)
            nc.scalar.activation(out=etc02_v[:, 2:], in_=tcols[:, 2:],
                                 func=AF.Exp, scale=NEG)
            nc.scalar.copy(out=wt_v[:, 1:, :, 0:32], in_=hn_v[:, 1:])
            wts = [wt_all[:, 132 * jt:132 * (jt + 1)] for jt in range(NT)]

        # num4[32h+d, i] = unnormalized out rows; zq row 32h = Z of head
        # h (rows between are dead lanes: engines can only address
        # 32-aligned partition offsets, and NR over the dead lanes is
        # free -- DVE cost is free-dim-driven. Memset keeps them 0 so
        # the K=97 broadcast matmul sees finite values under weight 0.)
        num4 = tpool.tile([128, N], f32, tag="num4")
        zq = tpool.tile([3 * HD + 1, N], f32, tag="zq")
        nc.gpsimd.memset(zq, 0.0)

        # ---- main loop: one tensor_scalar + two matmuls per (h, jt) ----
        with tc.tile_pool(name="ps_main", bufs=4, space="PSUM") as psmain:
            for h in range(H):
                oh = psmain.tile([33, N], f32, tag="oh")
                for jt in range(NT):
                    idx = h * NT + jt
                    a_t = apool.tile([128, N], adt, tag="at")
                    # all on DVE: gpsimd's tensor_scalar is ~30x slower
                    # and its SBUF traffic degrades concurrent DVE ops.
                    # The very first tile goes in halves behind the
                    # half-copies of u_b0.
                    spans = [(0, 512), (512, N)] if idx == 0 else [(0, N)]
                    for lo, hi in spans:
                        nc.vector.tensor_scalar(
                            out=a_t[:, lo:hi], in0=u_b[h][:, lo:hi],
                            scalar1=etc02[:, idx:idx + 1],
                            scalar2=etc[:, idx:idx + 1],
                            op0=ALU.mult, op1=ALU.max)
                    for c in range(2):
                        nc.tensor.matmul(
                            oh[:, 512 * c:512 * (c + 1)],
                            wts[jt][:, 33 * h:33 * (h + 1)],
                            a_t[:, 512 * c:512 * (c + 1)],
                            start=(jt == 0), stop=(jt == NT - 1))
                # per-head epilogue, pipelined with the next heads' bulk
                # work, on the otherwise-idle ACT engine: Z row straight
                # to partition 32h (32-aligned = legal), numerator rows
                # to num4 (frees the PSUM banks for the rzb matmuls)
                for c in range(2):
                    nc.scalar.copy(
                        out=zq[HD * h:HD * h + 1, 512 * c:512 * (c + 1)],
                        in_=oh[32:33, 512 * c:512 * (c + 1)])
                nc.scalar.copy(out=num4[HD * h:HD * (h + 1), :],
                               in_=oh[0:32, :])

        # ---- tail: nrz = -1/Z via int-magic seed + one Newton step
        # (xor and add cannot fuse: HW rejects mixed bitwise/arith) ----
        NP = 3 * HD + 1
        ynot = tpool.tile([NP, N], f32, tag="ynot")
        y = tpool.tile([NP, N], f32, tag="y")
        m = tpool.tile([NP, N], f32, tag="nr_m")
        # bf16 nrz: the rzb matmuls then stream at full bf16 rate with
        # overlapped weight loads; ~0.4% scale error vs the 2e-2 gate
        nrz = tpool.tile([NP, N], adt, tag="nrz")
        # per column half, so half 0's broadcast/multiply/DMA overlap
        # half 1's Newton chain
        for c in range(2):
            cs = slice(512 * c, 512 * (c + 1))
            nc.vector.tensor_scalar(
                out=ynot[:, cs].bitcast(i32), in0=zq[:, cs].bitcast(i32),
                scalar1=0xFFFFFFFF - (1 << 32), scalar2=None,
                op0=ALU.bitwise_xor)
            nc.vector.tensor_scalar(
                out=y[:, cs].bitcast(i32), in0=ynot[:, cs].bitcast(i32),
                scalar1=0x7EF311C4, scalar2=None, op0=ALU.add)
            nc.vector.tensor_tensor(out=m[:, cs], in0=zq[:, cs],
                                    in1=y[:, cs], op=ALU.mult)
            nc.vector.scalar_tensor_tensor(
                out=nrz[:, cs], in0=m[:, cs], scalar=2.0, in1=y[:, cs],
                op0=ALU.subtract, op1=ALU.mult)  # (m-2)*y = -1/Z
        with tc.tile_pool(name="ps_norm", bufs=1, space="PSUM") as psnorm:
            for c in range(2):
                # rzb[32h+d, i'] = nrz[32h, i'] via K=97 indicator matmul
                rzb = psnorm.tile([128, 512], f32, tag=f"rzb{c}")
                nc.tensor.matmul(rzb[:, :], ind97,
                                 nrz[:, 512 * c:512 * (c + 1)],
                                 start=True, stop=True)
                o_sb = tpool.tile([128, 512], adt, tag=f"osb{c}")
                nc.vector.scalar_tensor_tensor(
                    out=o_sb, in0=num4[:, 512 * c:512 * (c + 1)],
                    scalar=-1.0, in1=rzb, op0=ALU.mult, op1=ALU.mult)
                eng = nc.sync if c == 0 else nc.gpsimd
                eng.dma_start(out=outT[:, 512 * c:512 * (c + 1)], in_=o_sb)

    nc.compile()
    return nc


def _get_nc():
    if "nc" not in _CACHE:
        _CACHE["nc"] = _build_nc()
    return _CACHE["nc"]


def make_in_maps(x, W, a_src, a_dst):
    a_ext = np.zeros((OUT_F, 2 * H), np.float32)
    for h in range(H):
        a_ext[h * HD:(h + 1) * HD, h] = a_src[h]
        a_ext[h * HD:(h + 1) * HD, H + h] = a_dst[h]
    Wa = W @ a_ext
    # ind97[k, p] = 1 iff k == 32*(p//32) (Z-row broadcast), pre-cast
    # to bf16 so the device needs no conversion
    import ml_dtypes
    ind97 = np.zeros((3 * HD + 1, OUT_F), ml_dtypes.bfloat16)
    for h in range(H):
        ind97[HD * h, HD * h:HD * (h + 1)] = 1.0
    return [
        {"xT": np.ascontiguousarray(x[c].T), "W": W, "Wa": Wa,
         "ind97": ind97}
        for c in range(N_CORES)
    ]


def kernel(x, W, a_src, a_dst):
    from concourse.bass_utils import run_bass_kernel_spmd

    x = np.asarray(x, dtype=np.float32)
    W = np.asarray(W, dtype=np.float32)
    a_src = np.asarray(a_src, dtype=np.float32)
    a_dst = np.asarray(a_dst, dtype=np.float32)

    nc = _get_nc()
    in_maps = make_in_maps(x, W, a_src, a_dst)
    res = run_bass_kernel_spmd(nc, in_maps, core_ids=list(range(N_CORES)))
    out = np.stack([np.asarray(res.results[c]["outT"]).astype(np.float32).T
                    for c in range(N_CORES)], axis=0)
    return np.ascontiguousarray(out, dtype=np.float32)
